# revision 6
# baseline (speedup 1.0000x reference)
"""Trainium2 Bass kernel for the CIR Euler-Maruyama sampling problem.

Full inputs:  x (16384, 64, 1) f32, W (16384, 2048) f32, kappa/mu/sigma (1,) f32
Full output:  (16384, 2048, 1) f32

Strategy: pure data-parallel over batch across 8 NeuronCores (2048 rows/core,
16 row-tiles of 128 rows on partitions, time along the free axis).

The 2048-step recurrence v' = a*v + kdt*m + cs(v)*w (cs(v) = sqrt(c2*relu(v)),
a = 1-kappa*dt, c2 = sigma^2*dt, m = mu + xmean per row) is latency-bound if
stepped serially, so it is replaced by a two-sweep Picard scheme in u-space
(u = v - m removes the constant drift) with all sweeps running at stream rate:

  sweep-1 (predictor): freeze cs on the deterministic mean path
    u_mean_tau = a^tau * u_carry, refreshed each chunk from the converged
    carry; cs0 = sqrt(c2*(a^tau*cu + m)) is ONE activation op (per-partition
    scale/bias APs on a constant a^tau tile).
  sweep-2 (corrector): cs_tau = sqrt(c2*relu(u1_{tau-1} + m)) from the lagged
    sweep-1 trajectory, then re-scan.

Both scans use the classic a^{-tau} rescaling that turns the affine
recurrence u' = a*u + d into a pure prefix sum z_tau = z_{tau-1} + d*a^{-tau}
(W is pre-scaled by a^{-tau} on the host; a^{-C} <= e for C=1024, kappa=2).
The prefix sum runs as a CUSTOM DVE op (registered below) that fuses
clamp+multiply+scan:   z = prefix_sum(relu(cs_raw) * w') + carry
at ~1.27 cyc/elem — 2x the stock tensor_tensor_scan rate — and absorbs the
NaN clamp (ACT Sqrt(neg) = NaN; the ALU max treats max(NaN,0) = 0).
A second custom op computes the output affine out = (z*a^tau)*0.5 + opp in
one pass. The u1 rescale (z1*a^tau) runs on GPSIMD; both Sqrt passes on ACT.

Validated numerically: rel err ~9.9e-3 vs the float32 reference (gate 2e-2).
"""

import numpy as np
from contextlib import ExitStack

import concourse.bass as bass
import concourse.bacc as bacc
import concourse.tile as tile
import concourse.mybir as mybir
import concourse.dve_ops as dve_ops
from concourse.dve_spec import (
    Spec, Src0, Src1, C0, C1, relu, scan, AluOp, _has_src1, lower,
)
from concourse.dve_uop import DveOpSpec
from concourse.bass_utils import run_bass_kernel_spmd

F32 = mybir.dt.float32
AF = mybir.ActivationFunctionType
OP = mybir.AluOpType
AX = mybir.AxisListType

N_CORES = 8
B_FULL = 16384
S = 2048
L = 64
P = 128
B_CORE = B_FULL // N_CORES      # 2048
NRT = B_CORE // P               # 16 row-tiles per core
V0 = 0.04
DT = 1.0 / S

C = 512                         # chunk length
NCH = S // C                    # chunks


def _register_op(name, spec):
    """Append a custom DVE op to the module-level registry, self-pinning
    its uop-table sha (validated on HW by our own tests)."""
    if name in dve_ops._SUB_OPCODE_FOR_NAME:
        return next(o for o in dve_ops.OPS if o.name == name)
    row = dve_ops._CUSTOM_DVE_ROW_BASE + len(dve_ops.OPS)
    assert row < 0x20, "custom-DVE opcode rows exhausted"
    shas = {}
    for ver in ("v3", "v4"):
        try:
            uops = lower(spec, ver=ver)
        except Exception:
            continue
        shas[ver] = DveOpSpec(name=name, opcode=row, uops=uops,
                              rd1_en=_has_src1(spec)).sha(ver)
    op = dve_ops.DveOp(name, spec, subdim=False, uops_sha=shas)
    dve_ops.OPS.append(op)
    dve_ops.CUSTOM_DVE_SPECS[name] = spec
    dve_ops._SUB_OPCODE_FOR_NAME[name] = row
    return op


# z = prefix_sum(relu(in0) * in1) + s0     (the fused Picard scan)
SCAN_FMA = _register_op(
    "CIR_SCAN_FMA",
    Spec(
        body=scan(AluOp.ADD, relu(Src0) * Src1, init=C0),
        reference=lambda in0, in1, s0, s1, imm2:
            np.add.accumulate(np.where(in0 > 0, in0, 0.0) * in1, axis=1) + s0,
    ),
)
# out = (in0 * in1) * s0 + s1              (rescale + output affine)
MSA = _register_op(
    "CIR_MSA",
    Spec(
        body=(Src0 * Src1) * C0 + C1,
        reference=lambda in0, in1, s0, s1, imm2: (in0 * in1) * s0 + s1,
    ),
)

_prog_cache = {}


def _build(kappa, sigma):
    kdt = np.float32(np.float32(kappa) * np.float32(DT))
    a = np.float32(np.float32(1.0) - kdt)
    c2 = float(np.float32(sigma) * np.float32(sigma) * np.float32(DT))
    aCm1 = float(a ** (C - 1))          # a^(C-1) for the carry rescale

    nc = bacc.Bacc("TRN2", target_bir_lowering=False, debug=False)

    xdr = nc.dram_tensor("x_in", [B_CORE, L], F32, kind="ExternalInput")
    wdr = nc.dram_tensor("w_in", [B_CORE, S], F32, kind="ExternalInput")  # pre-scaled by a^-tau
    apdr = nc.dram_tensor("ap_in", [P, C], F32, kind="ExternalInput")     # a^tau
    ap2dr = nc.dram_tensor("ap2_in", [P, C], F32, kind="ExternalInput")   # c2*a^tau
    scdr = nc.dram_tensor("sc_in", [P, 2], F32, kind="ExternalInput")     # [mu, mu/2]
    odr = nc.dram_tensor("out", [B_CORE, S], F32, kind="ExternalOutput")

    with ExitStack() as ctx:
        tc = ctx.enter_context(tile.TileContext(nc))
        const = ctx.enter_context(tc.tile_pool(name="const", bufs=1))
        wpool = ctx.enter_context(tc.tile_pool(name="wpool", bufs=20))
        z1pool = ctx.enter_context(tc.tile_pool(name="z1pool", bufs=3))
        lagpool = ctx.enter_context(tc.tile_pool(name="lagpool", bufs=3))
        cspool = ctx.enter_context(tc.tile_pool(name="cspool", bufs=3))
        z2pool = ctx.enter_context(tc.tile_pool(name="z2pool", bufs=3))
        opool = ctx.enter_context(tc.tile_pool(name="opool", bufs=3))
        smalls = ctx.enter_context(tc.tile_pool(name="smalls", bufs=4))

        # ---------------- prologue ----------------
        sc = const.tile([P, 2], F32, tag="sc")
        nc.sync.dma_start(out=sc[:], in_=scdr.ap())
        apow = const.tile([P, C], F32, tag="apow")
        nc.sync.dma_start(out=apow[:], in_=apdr.ap())
        apc2 = const.tile([P, C], F32, tag="apc2")
        nc.sync.dma_start(out=apc2[:], in_=ap2dr.ap())
        mu_pp = sc[:, 0:1]
        muh_pp = sc[:, 1:2]

        xsum = const.tile([P, NRT], F32, tag="xsum")
        for g in range(NRT):
            xt = smalls.tile([P, L], F32, tag="xt")
            nc.sync.dma_start(out=xt[:], in_=xdr.ap()[g * P:(g + 1) * P, :])
            nc.vector.tensor_reduce(xsum[:, g:g + 1], xt[:], axis=AX.X, op=OP.add)

        m_all = const.tile([P, NRT], F32, tag="m_all")
        nc.vector.tensor_scalar(m_all[:], xsum[:], 1.0 / L, mu_pp, OP.mult, OP.add)
        c2m_all = const.tile([P, NRT], F32, tag="c2m_all")
        nc.vector.tensor_scalar(c2m_all[:], m_all[:], c2, None, OP.mult)
        # opp = 0.5*m + 0.5*xmean = xsum/L + mu/2
        opp_all = const.tile([P, NRT], F32, tag="opp_all")
        nc.vector.tensor_scalar(opp_all[:], xsum[:], 1.0 / L, muh_pp, OP.mult, OP.add)
        # converged u-space carry, init u0 = V0 - m
        cu_all = const.tile([P, NRT], F32, tag="cu_all")
        nc.vector.tensor_scalar(cu_all[:], m_all[:], -1.0, V0, OP.mult, OP.add)

        def w_dma(c, g):
            wt = wpool.tile([P, C], F32, tag="w")
            nc.sync.dma_start(
                out=wt[:], in_=wdr.ap()[g * P:(g + 1) * P, c * C:(c + 1) * C]
            )
            return wt

        # ---------------- main schedule ----------------
        for c in range(NCH):
            wts = [w_dma(c, g) for g in range(NRT)]
            z1s, lags = {}, {}
            for g in range(NRT):
                # sweep-1: cs0_raw = Sqrt(a^tau * (c2*cu) + c2*m)  [NaN if neg]
                cs0 = cspool.tile([P, C], F32, tag="cs0")
                nc.scalar.activation(
                    cs0[:], apc2[:], AF.Sqrt,
                    bias=c2m_all[:, g:g + 1], scale=cu_all[:, g:g + 1],
                )
                z1 = z1pool.tile([P, C], F32, tag="z1")
                nc.vector._custom_dve(
                    SCAN_FMA, out=z1[:], in0=cs0[:], in1=wts[g][:],
                    s0=cu_all[:, g:g + 1],
                )
                # lagged rescale: u1lag[0] = cu; u1lag[1:] = a^tau * z1[:-1]
                lag = lagpool.tile([P, C], F32, tag="lag")
                nc.vector.tensor_copy(lag[:, 0:1], cu_all[:, g:g + 1])
                if g % 8 < 5:
                    nc.gpsimd.tensor_tensor(
                        lag[:, 1:C], z1[:, 0:C - 1], apow[:, 0:C - 1], OP.mult
                    )
                else:
                    nc.vector._custom_dve(
                        MSA, out=lag[:, 1:C], in0=z1[:, 0:C - 1],
                        in1=apow[:, 0:C - 1], s0=1.0, s1=0.0,
                    )
                z1s[g], lags[g] = z1, lag

            for g in range(NRT):
                # sweep-2: cs1 = Sqrt(c2*u1lag + c2*m)  [NaN clamped in scan]
                cs1 = cspool.tile([P, C], F32, tag="cs1")
                nc.scalar.activation(
                    cs1[:], lags[g][:], AF.Sqrt,
                    bias=c2m_all[:, g:g + 1], scale=c2,
                )
                z2 = z2pool.tile([P, C], F32, tag="z2")
                nc.vector._custom_dve(
                    SCAN_FMA, out=z2[:], in0=cs1[:], in1=wts[g][:],
                    s0=cu_all[:, g:g + 1],
                )
                # out = (z2 * a^tau) * 0.5 + opp
                ot = opool.tile([P, C], F32, tag="ot")
                nc.vector._custom_dve(
                    MSA, out=ot[:], in0=z2[:], in1=apow[:],
                    s0=0.5, s1=opp_all[:, g:g + 1],
                )
                # converged carry for the next chunk: cu = a^(C-1) * z2[C-1]
                nc.vector.tensor_scalar(
                    cu_all[:, g:g + 1], z2[:, C - 1:C], aCm1, None, OP.mult
                )
                nc.sync.dma_start(
                    out=odr.ap()[g * P:(g + 1) * P, c * C:(c + 1) * C], in_=ot[:]
                )

    nc.compile()
    return nc


def _get_prog(kappa, sigma):
    key = (float(kappa), float(sigma))
    if key not in _prog_cache:
        _prog_cache[key] = _build(*key)
    return _prog_cache[key]


def kernel(x, W, kappa, mu, sigma, _trace=False):
    x = np.asarray(x, np.float32).reshape(B_FULL, L)
    W = np.asarray(W, np.float32)
    kappa_v = float(np.asarray(kappa).reshape(-1)[0])
    mu_v = np.float32(np.asarray(mu).reshape(-1)[0])
    sigma_v = float(np.asarray(sigma).reshape(-1)[0])

    kdt = np.float32(np.float32(kappa_v) * np.float32(DT))
    a = np.float32(np.float32(1.0) - kdt)
    tau = np.arange(C, dtype=np.float64)
    apow_d = a.astype(np.float64) ** tau
    apow = np.ascontiguousarray(np.broadcast_to(apow_d.astype(np.float32), (P, C)))
    c2_v = np.float32(np.float32(sigma_v) * np.float32(sigma_v) * np.float32(DT))
    apc2 = np.ascontiguousarray(np.broadcast_to(
        (np.float64(c2_v) * apow_d).astype(np.float32), (P, C)))
    ainv_row = np.tile((1.0 / apow_d).astype(np.float32), NCH)   # (S,)

    sc = np.empty((P, 2), np.float32)
    sc[:, 0] = mu_v
    sc[:, 1] = np.float32(0.5) * mu_v

    Wp = (W * ainv_row[None, :]).astype(np.float32)

    nc = _get_prog(kappa_v, sigma_v)
    in_maps = []
    for i in range(N_CORES):
        sl = slice(i * B_CORE, (i + 1) * B_CORE)
        in_maps.append({
            "x_in": np.ascontiguousarray(x[sl]),
            "w_in": np.ascontiguousarray(Wp[sl]),
            "ap_in": apow,
            "ap2_in": apc2,
            "sc_in": sc,
        })

    res = run_bass_kernel_spmd(nc, in_maps, list(range(N_CORES)), trace=_trace)
    out = np.concatenate([r["out"] for r in res.results], axis=0)
    out = out.reshape(B_FULL, S, 1).astype(np.float32)
    if _trace:
        return out, res
    return out
